# revision 5
# baseline (speedup 1.0000x reference)
"""DistMult decoder kernel for 8 Trainium2 NeuronCores.

Computes out = (input1 * weight[type_index]) @ input2.T + bias with
input1 [8192, 512], input2 [8192, 512] in fp32, out [8192, 8192].

Sharding: rows of input1 (and thus rows of the output) are split across
the 8 cores; input2 / weight / bias are replicated. No communication.

Per-core device program (M = 1024 rows):
  - lhsT  [512, 1024]  = w_r-scaled shard of input1, transposed + cast
    to fp16 on host (K-major); rhs [512, 8192] = input2 transposed +
    cast to fp16 on host.  fp16 runs the PE at 1 cycle/row with fp32
    PSUM accumulation; the whole rhs (64 KB/partition) + lhsT are SBUF
    resident so there is no mid-stream prefetch.
  - compute iterates 16 column slices of 512; all 8 m-tiles accumulate
    a slice in 8 PSUM banks, so one 512 KB rhs slice unlocks ~6.9 us
    of PE work and DMA arrival order matches compute order.
  - one dma_start executes on a single DMA channel (~22 GB/s), so the
    critical head loads are chopped into ~32-64 KB pieces spread over
    both HWDGE rings (sync + scalar) to engage many channels at once;
    the bulk (slices 4-15) rides the GpSimd SWDGE ring whose ~630 ns
    per-descriptor generation self-paces it ahead of consumption
    without stealing channels from the critical head.
  - output is stored as fp16 (16 MB/core instead of 32 MB) and upcast
    to fp32 on the host; stores alternate sync/scalar and the last
    slice's stores are chopped so the final drain spans many channels.
  - PSUM -> SBUF copy + bias add alternates between ACT and DVE.
  - warmup matmuls on uninitialized SBUF (no data deps at all) start
    the PE's HAM clock ramp right when the engines come up.
"""

import os

import numpy as np

import concourse.bacc as bacc
import concourse.mybir as mybir
from concourse.bass_utils import run_bass_kernel_spmd
from concourse.tile import TileContext

N_CORES = 8
N1, N2, D = 8192, 8192, 512
M = N1 // N_CORES  # rows per core
P = 128            # partitions
KT = D // P        # 4 k-tiles
MT = M // P        # 8 m-tiles
NFREE = 512        # psum bank free size (fp32)
NSL = N2 // NFREE  # 16 column slices

# test.py hooks: set TRACE=True before calling kernel() to profile; the
# BassKernelResults of the last run lands in LAST_RESULTS.
TRACE = os.environ.get("BASS_KERNEL_TRACE", "0") == "1"
LAST_RESULTS = None

_cached_nc = None


def _build():
    nc = bacc.Bacc(
        "TRN2", target_bir_lowering=False, debug=False, enable_asserts=False, num_devices=N_CORES
    )
    f32 = mybir.dt.float32
    f16 = mybir.dt.float16
    lhsT = nc.dram_tensor("lhsT", [D, M], f16, kind="ExternalInput")
    rhs = nc.dram_tensor("rhs", [D, N2], f16, kind="ExternalInput")
    biasv = nc.dram_tensor("biasv", [P, 1], f32, kind="ExternalInput")
    out = nc.dram_tensor("out", [M, N2], f16, kind="ExternalOutput")

    # K-major DRAM views split into [P, KT, cols] for single-DMA loads.
    lhsT_r = lhsT[:, :].rearrange("(kt p) m -> p kt m", p=P)
    rhs_r = rhs[:, :].rearrange("(kt p) n -> p kt n", p=P)

    with TileContext(nc) as tc:
        with (
            tc.tile_pool(name="const", bufs=1) as constp,
            tc.tile_pool(name="lhs", bufs=1) as lhsp,
            tc.tile_pool(name="rhsp", bufs=1) as rhsp,
            tc.tile_pool(name="outp", bufs=12) as outp,
            tc.tile_pool(name="psum", bufs=8, space="PSUM") as psump,
        ):
            lt = lhsp.tile([P, KT, M], f16, tag="lhs")
            rt = rhsp.tile([P, KT, N2], f16, tag="rhs")
            bias_t = constp.tile([P, 1], f32, tag="bias")

            # Alternate the two HWDGE rings piece-by-piece.
            rings = [nc.sync, nc.scalar]
            rr = [0]

            def ld(dst, src):
                rings[rr[0] % 2].dma_start(out=dst, in_=src)
                rr[0] += 1

            def ld_rt(s, k, c0, c1):
                ld(rt[:, k, s * NFREE + c0 : s * NFREE + c1],
                   rhs_r[:, k, s * NFREE + c0 : s * NFREE + c1])

            # Head, in exact consumption order, chopped fine so each
            # piece lands on its own DMA channel:
            #  1. bias; per k: lhsT m0-block (32 KB) + slice0 k in two
            #     64 KB halves  -> chain (s0, m0) streams immediately
            #  2. lhsT remainder (m1..m7) in 112 KB halves per k
            #  3. slice1 in 64 KB halves, slices 2-3 in 128 KB pieces
            #  4. slices 4-15 on the GpSimd SWDGE ring
            nc.sync.dma_start(out=bias_t[:], in_=biasv[:, :])
            for k in range(KT):
                ld(lt[:, k, 0:P], lhsT_r[:, k, 0:P])
                ld_rt(0, k, 0, 256)
                ld_rt(0, k, 256, 512)
            for k in range(KT):
                ld(lt[:, k, P:576], lhsT_r[:, k, P:576])
                ld(lt[:, k, 576:M], lhsT_r[:, k, 576:M])
            for k in range(KT):
                ld_rt(1, k, 0, 256)
                ld_rt(1, k, 256, 512)
            for s in (2, 3):
                for k in range(KT):
                    ld_rt(s, k, 0, 512)
            for s in range(4, NSL):
                for k in range(KT):
                    nc.gpsimd.dma_start(
                        out=rt[:, k, s * NFREE : (s + 1) * NFREE],
                        in_=rhs_r[:, k, s * NFREE : (s + 1) * NFREE],
                    )

            # Warm up the PE's HAM clock gate during the head-load
            # window: dummy matmuls push the PE through its ~3.4 us ramp
            # to 2.4 GHz while the loads land.
            warm_w = constp.tile([P, P], f16, tag="warmw")
            warm_r = constp.tile([P, NFREE], f16, tag="warmr")
            nc.vector.memset(warm_w[:], 0.0)
            nc.vector.memset(warm_r[:], 0.0)
            wps = psump.tile([P, NFREE], f32, tag="ps")
            NWARM = 10
            for i in range(NWARM):
                nc.tensor.matmul(
                    wps[:], warm_w[:], warm_r[:],
                    start=(i == 0), stop=(i == NWARM - 1),
                )

            for s in range(NSL):
                cols = slice(s * NFREE, (s + 1) * NFREE)
                for m in range(MT):
                    ps = psump.tile([P, NFREE], f32, tag="ps")
                    for k in range(KT):
                        nc.tensor.matmul(
                            ps[:], lt[:, k, m * P : (m + 1) * P],
                            rt[:, k, cols],
                            start=(k == 0), stop=(k == KT - 1),
                        )
                    ot = outp.tile([P, NFREE], f16, tag="ot")
                    # Alternate psum->sbuf+bias between ACT and the DVE so
                    # neither engine serializes the psum pool.
                    if m % 2 == 0:
                        nc.scalar.activation(
                            ot[:], ps[:],
                            mybir.ActivationFunctionType.Identity,
                            bias=bias_t[:, 0:1],
                        )
                    else:
                        nc.vector.tensor_scalar_add(ot[:], ps[:], bias_t[:, 0:1])
                    st = nc.sync if m % 2 == 0 else nc.scalar
                    orow = out[m * P : (m + 1) * P, cols]
                    if s == NSL - 1:
                        # Final slice: chop each store 4-ways so the exit
                        # drain runs on many DMA channels at once.
                        for c in range(0, NFREE, 128):
                            st.dma_start(
                                out=orow[:, c : c + 128],
                                in_=ot[:, c : c + 128],
                            )
                    else:
                        st.dma_start(out=orow, in_=ot[:])
    nc.compile()
    return nc


def kernel(input1, input2, weight, bias, type_index):
    global _cached_nc, LAST_RESULTS

    input1 = np.asarray(input1, dtype=np.float32)
    input2 = np.asarray(input2, dtype=np.float32)
    weight = np.asarray(weight, dtype=np.float32)
    bias = np.asarray(bias, dtype=np.float32).reshape(-1)
    w_r = weight[int(type_index)]  # [D]

    # Host-side prep: fold the w_r row-scale into input1, lay both GEMM
    # operands out K-major, cast to fp16 (device accumulates in fp32).
    scaled = input1 * w_r[None, :]  # [N1, D]
    rhsT = np.ascontiguousarray(input2.T.astype(np.float16))  # [D, N2]
    bias_vec = np.full((P, 1), float(bias[0]), dtype=np.float32)

    in_maps = []
    for c in range(N_CORES):
        shard = scaled[c * M : (c + 1) * M]  # [M, D]
        in_maps.append(
            {
                "lhsT": np.ascontiguousarray(shard.T.astype(np.float16)),
                "rhs": rhsT,
                "biasv": bias_vec,
            }
        )

    if _cached_nc is None:
        _cached_nc = _build()

    res = run_bass_kernel_spmd(
        _cached_nc, in_maps, core_ids=list(range(N_CORES)), trace=TRACE
    )
    LAST_RESULTS = res
    return np.concatenate(
        [res.results[c]["out"] for c in range(N_CORES)], axis=0
    ).astype(np.float32)
